# revision 1
# baseline (speedup 1.0000x reference)
"""Trainium2 Bass kernel for DAS (delay-and-sum) ultrasound beamforming.

Math: the per-(t,e,z) delay/phase depend on (t,e) only through
vx = gx[t]-ex[e], i.e. on delta = t-e (Toeplitz geometry). Per-delta tables
(gather index i0, fused interp/rotation/apod weights) are computed on host
from the small geometry inputs; the 512MB of sample data is processed on
8 NeuronCores:

  per (core, slot) = one delta diagonal: DMA the diagonal rows (t, t-delta)
  of interleaved I/Q data -> GPSIMD ap_gather at i0 and i0+1 (indices shared
  across partitions = transmits) -> PE transpose to [z, t] -> DVE/ACT
  multiply by per-delta weight columns (free-axis broadcast) and accumulate.
  Host sums the 8 per-core partial [z,t] accumulators.

The apodization mask is validated exactly per (t,e,z) on host; any mismatch
vs the delta-representative mask is fixed with sparse host corrections
(zero for the reference geometry).

SPMD uniformity: 255 deltas + 1 dummy = 256 (core,slot) instances arranged
in 32 slots x 8 cores, grouped by |delta| so every core's slot k has the
same compiled partition extent/offset.
"""
import os
import sys

for _p in ('/opt/trn_rl_repo', '/root/.axon_site/_ro/trn_rl_repo'):
    if os.path.isdir(_p) and _p not in sys.path:
        sys.path.append(_p)

import numpy as np

T, E, S, Z = 128, 128, 4096, 2048
PI = 3.14159265359
MIN_WIDTH = 0.001
N_CORES = 8
N_SLOTS = 32
NBLK = 16          # z blocks of 128
DUMMY = 999


def _f32(x):
    return np.asarray(x, dtype=np.float32)


# ---------------------------------------------------------------- host math
def build_slot_assignment():
    pos = sorted(range(0, 128), key=lambda d: -d)
    neg = [DUMMY] + sorted(range(-127, 0), key=lambda d: d)
    slots = []
    for k in range(16):
        group = pos[8 * k: 8 * k + 8]
        toff = min(group)
        slots.append(dict(toff=toff, ext=128 - toff, deltas=group))
    for k in range(16):
        group = neg[8 * k: 8 * k + 8]
        real = [d for d in group if d != DUMMY]
        ext = 128 - min(abs(d) for d in real)
        slots.append(dict(toff=0, ext=ext, deltas=group))
    return slots


def compute_tables(grid, tx_ori, ele_pos, time_zero, fs, c, fdemod, rxfnum):
    grid = _f32(grid); tx_ori = _f32(tx_ori); ele_pos = _f32(ele_pos)
    time_zero = _f32(time_zero)
    gx = grid[:, 0, 0]
    zax = grid[0, :, 2]
    ex = ele_pos[:, 0]

    vx_te = (gx[:, None] - ex[None, :]).astype(np.float32)
    vz = zax.astype(np.float32)
    with np.errstate(divide='ignore', invalid='ignore'):
        ratio = np.abs(vz[None, None, :] / vx_te[:, :, None])
    m = ratio > np.float32(rxfnum)
    m |= (np.abs(vx_te) <= np.float32(MIN_WIDTH))[:, :, None]
    m |= ((vx_te >= np.float32(MIN_WIDTH)) & (gx[:, None] <= ex[0]))[:, :, None]
    m |= ((vx_te <= np.float32(-MIN_WIDTH)) & (gx[:, None] >= ex[-1]))[:, :, None]
    mask_exact = m

    d3 = grid - tx_ori[:, None, :]
    txdel = np.sqrt((d3 * d3).sum(-1, dtype=np.float32)).astype(np.float32)

    nd = 255
    i0_tab = np.zeros((nd, Z), np.int32)
    frac_tab = np.zeros((nd, Z), np.float32)
    ct_tab = np.zeros((nd, Z), np.float32)
    st_tab = np.zeros((nd, Z), np.float32)
    v0_tab = np.zeros((nd, Z), np.float32)
    v1_tab = np.zeros((nd, Z), np.float32)
    mask_tab = np.zeros((nd, Z), bool)
    for delta in range(-127, 128):
        t_rep = max(0, delta); e_rep = t_rep - delta
        vx = vx_te[t_rep, e_rep]
        rx = np.sqrt(vx * vx + vz * vz).astype(np.float32)
        delays = ((txdel[t_rep] + rx) / np.float32(c)
                  + time_zero[t_rep]) * np.float32(fs)
        i0f = np.floor(delays)
        frac = (delays - i0f).astype(np.float32)
        i0 = i0f.astype(np.int32)
        tshift = delays / np.float32(fs) - zax * np.float32(2.0) / np.float32(c)
        theta = (np.float32(2.0 * PI * fdemod) * tshift).astype(np.float32)
        j = delta + 127
        i0_tab[j] = i0
        frac_tab[j] = frac
        ct_tab[j] = np.cos(theta, dtype=np.float32)
        st_tab[j] = np.sin(theta, dtype=np.float32)
        v0_tab[j] = (i0 >= 0) & (i0 < S)
        v1_tab[j] = (i0 + 1 >= 0) & (i0 + 1 < S)
        mask_tab[j] = mask_exact[t_rep, e_rep]
    return dict(i0=i0_tab, frac=frac_tab, ct=ct_tab, st=st_tab,
                v0=v0_tab, v1=v1_tab, mask_tab=mask_tab,
                mask_exact=mask_exact)


def build_weight_tables(tabs):
    """[255, 6, Z]: (wa, wb, -wc, -wd, wc, wd);
    accI += wa*I0 + wb*I1 - wc*Q0 - wd*Q1
    accQ += wc*I0 + wd*I1 + wa*Q0 + wb*Q1"""
    apod = tabs['mask_tab'].astype(np.float32)
    omf = np.float32(1.0) - tabs['frac']
    wa = apod * tabs['ct'] * omf * tabs['v0']
    wb = apod * tabs['ct'] * tabs['frac'] * tabs['v1']
    wc = apod * tabs['st'] * omf * tabs['v0']
    wd = apod * tabs['st'] * tabs['frac'] * tabs['v1']
    return np.stack([wa, wb, -wc, -wd, wc, wd], axis=1).astype(np.float32)


def corrections(idata, qdata, tabs):
    corrI = np.zeros((T, Z), np.float32)
    corrQ = np.zeros((T, Z), np.float32)
    i0c = np.clip(tabs['i0'], 0, S - 1)
    i1c = np.clip(tabs['i0'] + 1, 0, S - 1)
    for delta in range(-127, 128):
        j = delta + 127
        ts = np.arange(max(0, delta), min(T - 1, T - 1 + delta) + 1)
        es = ts - delta
        dm = (tabs['mask_exact'][ts, es, :].astype(np.int8)
              - tabs['mask_tab'][j][None, :].astype(np.int8))
        nz = np.argwhere(dm != 0)
        if nz.size == 0:
            continue
        ti, zi = nz[:, 0], nz[:, 1]
        tt, ee = ts[ti], es[ti]
        sgn = dm[ti, zi].astype(np.float32)
        f = tabs['frac'][j][zi]; ct = tabs['ct'][j][zi]; st = tabs['st'][j][zi]
        v0 = tabs['v0'][j][zi]; v1 = tabs['v1'][j][zi]
        I0 = idata[tt, ee, i0c[j][zi]] * v0; I1 = idata[tt, ee, i1c[j][zi]] * v1
        Q0 = qdata[tt, ee, i0c[j][zi]] * v0; Q1 = qdata[tt, ee, i1c[j][zi]] * v1
        fi = (1 - f) * I0 + f * I1
        fq = (1 - f) * Q0 + f * Q1
        np.add.at(corrI, (tt, zi), sgn * (ct * fi - st * fq))
        np.add.at(corrQ, (tt, zi), sgn * (ct * fq + st * fi))
    return corrI, corrQ


# ------------------------------------------------------------- bass program
_CACHE = {}


def _build_program(slots):
    import concourse.bacc as bacc
    import concourse.mybir as mybir
    from concourse.tile import TileContext
    from concourse.masks import make_identity

    DT = mybir.dt.float32
    r_tot = sum(sl['ext'] for sl in slots)
    nc = bacc.Bacc("TRN2", target_bir_lowering=False, debug=False,
                   num_devices=N_CORES)
    rows_d = nc.dram_tensor("rows", [r_tot, S * 2], DT, kind="ExternalInput").ap()
    idx_d = nc.dram_tensor("idx", [N_SLOTS, 128, 256], mybir.dt.int16,
                           kind="ExternalInput").ap()
    wts_d = nc.dram_tensor("wts", [N_SLOTS, 128, 96], DT,
                           kind="ExternalInput").ap()
    accI_d = nc.dram_tensor("accI", [128, Z], DT, kind="ExternalOutput").ap()
    accQ_d = nc.dram_tensor("accQ", [128, Z], DT, kind="ExternalOutput").ap()

    with TileContext(nc) as tc:
        with tc.tile_pool(name="data", bufs=2) as dpool, \
             tc.tile_pool(name="gout", bufs=2) as gpool, \
             tc.tile_pool(name="small", bufs=2) as spool, \
             tc.tile_pool(name="tmp", bufs=3) as tpool, \
             tc.tile_pool(name="accp", bufs=1) as apool, \
             tc.tile_pool(name="psum", bufs=2, space="PSUM") as ppool:
            ident = apool.tile([128, 128], DT, tag="ident")
            make_identity(nc, ident[:])
            accI = apool.tile([128, NBLK, 128], DT, tag="accI")
            accQ = apool.tile([128, NBLK, 128], DT, tag="accQ")
            nc.vector.memset(accI[:], 0.0)
            nc.vector.memset(accQ[:], 0.0)

            row_off = 0
            for k, sl in enumerate(slots):
                ext, toff = sl['ext'], sl['toff']
                data_t = dpool.tile([128, S, 2], DT, tag="data")
                nc.sync.dma_start(out=data_t[0:ext],
                                  in_=rows_d[row_off:row_off + ext])
                idx_t = spool.tile([128, 256], mybir.dt.int16, tag="idx")
                nc.sync.dma_start(out=idx_t[:], in_=idx_d[k])
                w_t = spool.tile([128, 96], DT, tag="wts")
                nc.sync.dma_start(out=w_t[:], in_=wts_d[k])

                gout0 = gpool.tile([128, Z, 2], DT, tag="g0")
                gout1 = gpool.tile([128, Z, 2], DT, tag="g1")
                nc.gpsimd.ap_gather(gout0[:], data_t[:], idx_t[:, 0:128],
                                    channels=128, num_elems=S, d=2,
                                    num_idxs=Z)
                nc.gpsimd.ap_gather(gout1[:], data_t[:], idx_t[:, 128:256],
                                    channels=128, num_elems=S, d=2,
                                    num_idxs=Z)

                # (source tile, IQ channel, accI table idx, accQ table idx)
                for (src, ch, tabI, tabQ) in ((gout0, 0, 0, 4),
                                              (gout1, 0, 1, 5),
                                              (gout0, 1, 2, 0),
                                              (gout1, 1, 3, 1)):
                    big = ppool.tile([128, NBLK, 128], DT, space="PSUM",
                                     tag="big")
                    for blk in range(NBLK):
                        nc.tensor.transpose(
                            out=big[:, blk, :],
                            in_=src[:, blk * 128:(blk + 1) * 128, ch],
                            identity=ident[:])
                    for (acc, tab) in ((accI, tabI), (accQ, tabQ)):
                        w_ap = w_t[:, tab * 16:(tab + 1) * 16] \
                            .broadcast_to([128, NBLK, ext])
                        tmp = tpool.tile([128, NBLK, 128], DT, tag="tmp")
                        nc.any.tensor_tensor(
                            out=tmp[:, :, 0:ext], in0=big[:, :, 0:ext],
                            in1=w_ap, op=mybir.AluOpType.mult)
                        nc.any.tensor_tensor(
                            out=acc[:, :, toff:toff + ext],
                            in0=acc[:, :, toff:toff + ext],
                            in1=tmp[:, :, 0:ext], op=mybir.AluOpType.add)
                row_off += ext

            nc.sync.dma_start(out=accI_d[:], in_=accI[:])
            nc.sync.dma_start(out=accQ_d[:], in_=accQ[:])
    nc.compile()
    return nc


def _get_program_and_slots():
    if 'prog' not in _CACHE:
        slots = build_slot_assignment()
        _CACHE['slots'] = slots
        _CACHE['prog'] = _build_program(slots)
    return _CACHE['prog'], _CACHE['slots']


def _pack_inputs(idata, qdata, tabs, wtabs, slots):
    """Per-core input dicts."""
    data_iq = np.empty((T * E, S * 2), np.float32)
    data_iq[:, 0::2] = idata.reshape(T * E, S)
    data_iq[:, 1::2] = qdata.reshape(T * E, S)

    i0c = np.clip(tabs['i0'], 0, S - 1).astype(np.int16)
    i1c = np.clip(tabs['i0'] + 1, 0, S - 1).astype(np.int16)
    # wrapped idx layout: wrapped[p, s] = idx[s*16 + p%16]
    pp = (np.arange(128)[:, None] % 16)
    ss = np.arange(128)[None, :] * 16
    wrap_sel = (ss + pp)                      # [128,128]

    r_tot = sum(sl['ext'] for sl in slots)
    in_maps = []
    for c in range(N_CORES):
        rows = np.zeros((r_tot, S * 2), np.float32)
        idx = np.zeros((N_SLOTS, 128, 256), np.int16)
        wts = np.zeros((N_SLOTS, 128, 96), np.float32)
        row_off = 0
        for k, sl in enumerate(slots):
            ext, toff = sl['ext'], sl['toff']
            delta = sl['deltas'][c]
            if delta != DUMMY:
                j = delta + 127
                if delta >= 0:
                    ts = np.arange(delta, T)
                else:
                    ts = np.arange(0, T + delta)
                ps = ts - toff
                rows[row_off + ps] = data_iq[ts * E + (ts - delta)]
                idx[k, :, 0:128] = i0c[j][wrap_sel]
                idx[k, :, 128:256] = i1c[j][wrap_sel]
                # wts[k, p, tab*16+blk] = wtabs[j, tab, blk*128+p]
                wts[k] = wtabs[j].reshape(6, NBLK, 128) \
                    .transpose(2, 0, 1).reshape(128, 96)
            row_off += ext
        in_maps.append({"rows": rows, "idx": idx, "wts": wts})
    return in_maps


def kernel(idata, qdata, grid, tx_ori, ele_pos, time_zero,
           fs, c, fdemod, rxfnum):
    from concourse.bass_utils import run_bass_kernel_spmd

    idata = _f32(idata); qdata = _f32(qdata)
    tabs = compute_tables(grid, tx_ori, ele_pos, time_zero,
                          fs, c, fdemod, rxfnum)
    wtabs = build_weight_tables(tabs)
    nc, slots = _get_program_and_slots()
    in_maps = _pack_inputs(idata, qdata, tabs, wtabs, slots)
    res = run_bass_kernel_spmd(nc, in_maps, list(range(N_CORES)))
    _CACHE['last_results'] = res

    idas = np.zeros((T, Z), np.float32)
    qdas = np.zeros((T, Z), np.float32)
    for cidx in range(N_CORES):
        aI = res.results[cidx]["accI"].reshape(128, NBLK, 128)
        aQ = res.results[cidx]["accQ"].reshape(128, NBLK, 128)
        idas += aI.transpose(1, 0, 2).reshape(Z, T).T
        qdas += aQ.transpose(1, 0, 2).reshape(Z, T).T
    cI, cQ = corrections(idata, qdata, tabs)
    idas += cI
    qdas += cQ
    return (idas, qdas)

